# revision 18
# baseline (speedup 1.0000x reference)
"""AttentionalPropagation (SuperGlue-style MLP attention) Trainium2 kernel.

Full inputs in, full outputs out. Internally shards the nq (query) dimension
across 8 NeuronCores; source/keys and all MLP weights are replicated.

Math (per core, 64-query shard):
  aq = w1q @ q + b1            [256, 64]
  ak = w1k @ source            [256, 512]
  scores[n,m] = w2 . relu(aq[:,n] + ak[:,m]) + b2     (pairwise MLP score)
  prob = softmax_m(scores); message = source @ prob^T
  y = wb @ relu(wa @ [x; message] + ba) + bb

Implementation notes:
  - The pairwise tensor H_q = relu(ak + aq[:,q]) ([128, 512] per query per
    d-half, bf16) is built with single fused ops split across VectorE
    (tensor_scalar add+max), ScalarE (activation Relu+bias) and GpSimd.
  - The d-reduction (w2 . H) runs on the TensorEngine in bf16. To get score
    rows DENSELY packed in PSUM (DMA can't read PSUM and DVE/ACT cost is
    free-dim based), query r of a 16-query group uses lhsT = [0,..,0,w2half]
    (r+1 cols) so its row lands at partition base+r; zero columns add 0 to
    already-accumulated rows. 4 column-groups (tile_position) overlap on PE.
  - Softmax skips max-subtraction (scores are O(1), exp can't overflow).
  - prob (bf16) is transposed on the PE and the dense query columns gathered
    with a strided free-dim DVE copy; message + outer MLP are bf16 matmuls.
"""

import numpy as np
import ml_dtypes

import concourse.bass as bass
import concourse.bacc as bacc
import concourse.tile as tile
from concourse import mybir
from concourse.bass_utils import run_bass_kernel_spmd

D = 256
NQ = 512
NK = 512
NCORES = 8
QSH = NQ // NCORES      # queries per core = 64
NGRP = 4                # PE column groups
GQ = QSH // NGRP        # queries per group = 16
F32 = mybir.dt.float32
BF16 = mybir.dt.bfloat16

# construction engine pattern (per op index): D=vector, A=scalar, G=gpsimd
_ENG_PATTERN = "DDADDA"


def _build_program():
    nc = bacc.Bacc(trn_type="TRN2")

    # ---- DRAM parameters (per-core views; same program on all cores) ----
    d_src = nc.declare_dram_parameter("source", [D, NK], BF16, isOutput=False)
    d_w1kT = nc.declare_dram_parameter("w1kT", [D, D], BF16, isOutput=False)
    d_x = nc.declare_dram_parameter("x_shard", [D, QSH], BF16, isOutput=False)
    d_w1qT = nc.declare_dram_parameter("w1qT", [D, D], BF16, isOutput=False)
    d_cf32 = nc.declare_dram_parameter("constf32", [128, 9], F32, isOutput=False)
    d_cbf = nc.declare_dram_parameter("constbf16", [128, 2 * GQ + 128], BF16, isOutput=False)
    d_srcT = nc.declare_dram_parameter("sourceT", [NK, D], BF16, isOutput=False)
    d_waT = nc.declare_dram_parameter("waT", [2 * D, 2 * D], BF16, isOutput=False)
    d_wbT = nc.declare_dram_parameter("wbT", [2 * D, D], BF16, isOutput=False)
    d_scores = nc.declare_dram_parameter("scores_sh", [QSH, NK], F32, isOutput=True)
    d_y = nc.declare_dram_parameter("y_sh", [D, QSH], F32, isOutput=True)

    with tile.TileContext(nc) as tc:
        with (
            tc.tile_pool(name="consts", bufs=1) as consts,
            tc.tile_pool(name="acts", bufs=1) as acts,
            tc.tile_pool(name="hpool", bufs=24) as hpool,
            tc.tile_pool(name="psc", bufs=4, space="PSUM") as psc,
            tc.tile_pool(name="pmisc", bufs=4, space="PSUM") as pmisc,
        ):
            # ---------------- input loads (dependency order) ----------------
            src_sb = [consts.tile([128, NK], BF16, tag=f"src{c}", name=f"src{c}")
                      for c in range(2)]
            srcr = d_src[:].rearrange("(c p) m -> c p m", p=128)
            w1kT_sb = [consts.tile([128, D], BF16, tag=f"w1k{c}", name=f"w1k{c}")
                       for c in range(2)]
            w1kr = d_w1kT[:].rearrange("(c p) o -> c p o", p=128)
            for c in range(2):
                nc.sync.dma_start(out=w1kT_sb[c], in_=w1kr[c])
            for mh in range(2):
                for c in range(2):
                    nc.sync.dma_start(
                        out=src_sb[c][:, 256 * mh : 256 * (mh + 1)],
                        in_=srcr[c][:, 256 * mh : 256 * (mh + 1)],
                    )

            # cat = [x (2 chunks); message (2 chunks)]  as [128, 4, 64] bf16
            cat_sb = acts.tile([128, 4, QSH], BF16)
            xr = d_x[:].rearrange("(c p) n -> c p n", p=128)
            for c in range(2):
                nc.sync.dma_start(out=cat_sb[:, c, :], in_=xr[c])

            w1qT_sb = [consts.tile([128, D], BF16, tag=f"w1q{c}", name=f"w1q{c}")
                       for c in range(2)]
            w1qr = d_w1qT[:].rearrange("(c p) o -> c p o", p=128)
            for c in range(2):
                nc.sync.dma_start(out=w1qT_sb[c], in_=w1qr[c])

            # merged per-partition constants: cols 0-1 b1, 2 b2, 3-6 ba, 7-8 bb
            cf32_sb = consts.tile([128, 9], F32)
            nc.sync.dma_start(out=cf32_sb, in_=d_cf32[:])
            b1_sb = cf32_sb[:, 0:2]
            b2_sb = cf32_sb[:, 2:3]
            ba_sb = cf32_sb[:, 3:7]
            bb_sb = cf32_sb[:, 7:9]
            # merged bf16 block: [w2pad (2*GQ) | eye128 (128)]
            cbf_sb = consts.tile([128, 2 * GQ + 128], BF16)
            nc.gpsimd.dma_start(out=cbf_sb, in_=d_cbf[:])
            w2p_sb = cbf_sb[:, 0 : 2 * GQ].rearrange("p (h g) -> p h g", h=2)
            eye_sb = cbf_sb[:, 2 * GQ : 2 * GQ + 128]

            srcT_sb = [consts.tile([128, D], BF16, tag=f"srcT{t}", name=f"srcT{t}")
                       for t in range(4)]
            srcTr = d_srcT[:].rearrange("(t p) d -> t p d", p=128)
            for t in range(4):
                nc.gpsimd.dma_start(out=srcT_sb[t], in_=srcTr[t])

            waT_sb = [consts.tile([128, 2 * D], BF16, tag=f"waT{c}", name=f"waT{c}")
                      for c in range(4)]
            waTr = d_waT[:].rearrange("(c p) o -> c p o", p=128)
            for c in range(4):
                nc.gpsimd.dma_start(out=waT_sb[c], in_=waTr[c])

            wbT_sb = [consts.tile([128, D], BF16, tag=f"wbT{c}", name=f"wbT{c}")
                      for c in range(4)]
            wbTr = d_wbT[:].rearrange("(c p) o -> c p o", p=128)
            for c in range(4):
                nc.gpsimd.dma_start(out=wbT_sb[c], in_=wbTr[c])

            # ---------------- ak / aq ----------------
            ak_sb = [acts.tile([128, NK], BF16, tag=f"ak{h}", name=f"ak{h}")
                     for h in range(2)]
            for oc in range(2):
                p_ak = pmisc.tile([128, NK], F32, tag="pmisc")
                for mh in range(2):
                    for kc in range(2):
                        nc.tensor.matmul(
                            out=p_ak[:, 256 * mh : 256 * (mh + 1)],
                            lhsT=w1kT_sb[kc][:, 128 * oc : 128 * (oc + 1)],
                            rhs=src_sb[kc][:, 256 * mh : 256 * (mh + 1)],
                            start=(mh == 0 and kc == 0),
                            stop=(mh == 1 and kc == 1),
                            skip_group_check=True,
                        )
                nc.scalar.copy(out=ak_sb[oc][:], in_=p_ak[:])

            aq_sb = [acts.tile([128, QSH], F32, tag=f"aq{h}", name=f"aq{h}")
                     for h in range(2)]
            for oc in range(2):
                p_aq = pmisc.tile([128, QSH], F32, tag="pmisc")
                for kc in range(2):
                    nc.tensor.matmul(
                        out=p_aq[:],
                        lhsT=w1qT_sb[kc][:, 128 * oc : 128 * (oc + 1)],
                        rhs=cat_sb[:, kc, :],
                        start=(kc == 0),
                        stop=(kc == 1),
                    )
                nc.vector.tensor_scalar(
                    out=aq_sb[oc][:],
                    in0=p_aq[:],
                    scalar1=b1_sb[:, oc : oc + 1],
                    scalar2=None,
                    op0=mybir.AluOpType.add,
                )

            # ---------------- pairwise scores ----------------
            # sc_sb row 32*g + r holds scores for query q = 16*g + r
            sc_sb = acts.tile([128, NK], F32)

            p_sc = [psc.tile([128, NK], F32, tag="psc", name=f"psc{g}")
                    for g in range(NGRP)]

            opi = 0
            for r in range(GQ - 1, -1, -1):
                for h in range(2):
                    hts = []
                    for g in range(NGRP):
                        q = GQ * g + r
                        ht = hpool.tile([128, NK], BF16, tag="H", name=f"H{opi}")
                        eng = _ENG_PATTERN[opi % len(_ENG_PATTERN)]
                        if eng == "A":
                            nc.scalar.activation(
                                out=ht[:],
                                in_=ak_sb[h][:],
                                func=mybir.ActivationFunctionType.Relu,
                                bias=aq_sb[h][:, q : q + 1],
                            )
                        elif eng == "G":
                            nc.gpsimd.tensor_scalar(
                                out=ht[:],
                                in0=ak_sb[h][:],
                                scalar1=aq_sb[h][:, q : q + 1],
                                scalar2=0.0,
                                op0=mybir.AluOpType.add,
                                op1=mybir.AluOpType.max,
                            )
                        else:
                            nc.vector.tensor_scalar(
                                out=ht[:],
                                in0=ak_sb[h][:],
                                scalar1=aq_sb[h][:, q : q + 1],
                                scalar2=0.0,
                                op0=mybir.AluOpType.add,
                                op1=mybir.AluOpType.max,
                            )
                        opi += 1
                        hts.append(ht)
                    for g in range(NGRP):
                        nc.tensor.matmul(
                            out=p_sc[g][32 * g : 32 * g + r + 1, :],
                            lhsT=w2p_sb[:, h, GQ - 1 - r : GQ],
                            rhs=hts[g][:],
                            start=(r == GQ - 1 and h == 0),
                            stop=(r == 0 and h == 1),
                            tile_position=(0, 32 * g),
                            skip_group_check=True,
                        )

            # fused per-group extraction: e = exp(scores + b2), accum -> sums
            e_sb = acts.tile([128, NK], F32)
            sums = acts.tile([128, 1], F32)
            nc.gpsimd.memset(e_sb, 0.0)
            nc.gpsimd.memset(sums, 1.0)
            for g in range(NGRP):
                lo = 32 * g
                nc.scalar.activation(
                    out=e_sb[lo : lo + GQ, :],
                    in_=p_sc[g][lo : lo + GQ, :],
                    func=mybir.ActivationFunctionType.Exp,
                    bias=b2_sb[lo : lo + GQ, :],
                    accum_out=sums[lo : lo + GQ, :],
                )

            # MLP1 x-part: h2 += waT[0:2].T @ x while softmax runs
            p_h2 = [psc.tile([128, QSH], F32, tag="psc", name=f"ph2{oc}")
                    for oc in range(4)]
            for oc in range(4):
                for kc in range(2):
                    nc.tensor.matmul(
                        out=p_h2[oc][:],
                        lhsT=waT_sb[kc][:, 128 * oc : 128 * (oc + 1)],
                        rhs=cat_sb[:, kc, :],
                        start=(kc == 0),
                        stop=False,
                        skip_group_check=True,
                    )

            # ---------------- softmax (no max-subtraction) ----------------
            rec = acts.tile([128, 1], F32)
            nc.vector.reciprocal(out=rec[:], in_=sums[:])
            prob = acts.tile([128, NK], BF16)
            nc.vector.tensor_scalar(
                out=prob[:], in0=e_sb[:], scalar1=rec[:, 0:1], scalar2=None,
                op0=mybir.AluOpType.mult,
            )

            # raw scores to DRAM (off the critical path)
            for g in range(NGRP):
                lo = 32 * g
                nc.scalar.activation(
                    out=sc_sb[lo : lo + GQ, :],
                    in_=p_sc[g][lo : lo + GQ, :],
                    func=mybir.ActivationFunctionType.Identity,
                    bias=b2_sb[lo : lo + GQ, :],
                )
                nc.sync.dma_start(
                    out=d_scores[:][GQ * g : GQ * (g + 1), :],
                    in_=sc_sb[lo : lo + GQ, :],
                )

            # ---------------- transpose prob, gather dense columns ----------
            probT = [acts.tile([128, NGRP, GQ], BF16, tag=f"pT{t}", name=f"pT{t}")
                     for t in range(4)]
            for mt in range(4):
                p_t = pmisc.tile([128, 128], BF16, tag="pmisc")
                nc.tensor.transpose(
                    out=p_t[:], in_=prob[:, 128 * mt : 128 * (mt + 1)],
                    identity=eye_sb[:],
                )
                nc.vector.tensor_copy(
                    out=probT[mt][:],
                    in_=p_t[:].rearrange("p (g x) -> p g x", g=4)[:, :, 0:GQ],
                )

            # ---------------- message = source @ prob^T ----------------
            for oc in range(2):
                p_msg = pmisc.tile([128, QSH], F32, tag="pmisc")
                for mtile in range(4):
                    nc.tensor.matmul(
                        out=p_msg[:],
                        lhsT=srcT_sb[mtile][:, 128 * oc : 128 * (oc + 1)],
                        rhs=probT[mtile][:],
                        start=(mtile == 0),
                        stop=(mtile == 3),
                    )
                nc.vector.tensor_copy(out=cat_sb[:, 2 + oc, :], in_=p_msg[:])

            # ---------------- outer MLP (message part accumulates) ----------
            h2_sb = acts.tile([128, 4, QSH], BF16)
            for oc in range(4):
                for kc in range(2, 4):
                    nc.tensor.matmul(
                        out=p_h2[oc][:],
                        lhsT=waT_sb[kc][:, 128 * oc : 128 * (oc + 1)],
                        rhs=cat_sb[:, kc, :],
                        start=False,
                        stop=(kc == 3),
                        skip_group_check=True,
                    )
                nc.vector.tensor_scalar(
                    out=h2_sb[:, oc, :], in0=p_h2[oc][:],
                    scalar1=ba_sb[:, oc : oc + 1], scalar2=0.0,
                    op0=mybir.AluOpType.add, op1=mybir.AluOpType.max,
                )

            yr = d_y[:].rearrange("(c p) n -> c p n", p=128)
            for oc in range(2):
                p_y = pmisc.tile([128, QSH], F32, tag="pmisc")
                for kc in range(4):
                    nc.tensor.matmul(
                        out=p_y[:],
                        lhsT=wbT_sb[kc][:, 128 * oc : 128 * (oc + 1)],
                        rhs=h2_sb[:, kc, :],
                        start=(kc == 0),
                        stop=(kc == 3),
                    )
                y_sb = acts.tile([128, QSH], F32, tag=f"y{oc}", name=f"ysb{oc}")
                nc.vector.tensor_scalar(
                    out=y_sb[:], in0=p_y[:], scalar1=bb_sb[:, oc : oc + 1],
                    scalar2=None, op0=mybir.AluOpType.add,
                )
                nc.sync.dma_start(out=yr[oc], in_=y_sb[:])

    nc.finalize()
    return nc


_NC_CACHE = None


def _get_program():
    global _NC_CACHE
    if _NC_CACHE is None:
        _NC_CACHE = _build_program()
    return _NC_CACHE


def _prepare_in_maps(x, source, w1, b1, w2, b2, wa, ba, wb, bb):
    f = np.float32
    bf = ml_dtypes.bfloat16
    x = np.asarray(x, f)[0]          # [D, NQ]
    src = np.asarray(source, f)[0]   # [D, NK]
    w1 = np.asarray(w1, f)
    b1 = np.asarray(b1, f)
    w2 = np.asarray(w2, f)
    b2 = np.asarray(b2, f)
    wa = np.asarray(wa, f)
    ba = np.asarray(ba, f)
    wb = np.asarray(wb, f)
    bb = np.asarray(bb, f)

    w2pad = np.zeros((128, 2, GQ), bf)
    for h in range(2):
        w2pad[:, h, GQ - 1] = w2[0, 128 * h : 128 * (h + 1)].astype(bf)
    # merged consts: f32 [128, 9] = [b1(2) | b2 | ba(4) | bb(2)]
    cf32 = np.zeros((128, 9), f)
    cf32[:, 0] = b1[:128]
    cf32[:, 1] = b1[128:]
    cf32[:, 2] = b2[0]
    for c in range(4):
        cf32[:, 3 + c] = ba[128 * c : 128 * (c + 1)]
    cf32[:, 7] = bb[:128]
    cf32[:, 8] = bb[128:]
    # merged bf16 [128, 2*GQ + 128] = [w2pad flat | eye128]
    cbf = np.zeros((128, 2 * GQ + 128), bf)
    cbf[:, : 2 * GQ] = w2pad.reshape(128, 2 * GQ)
    cbf[:, 2 * GQ :] = np.eye(128, dtype=bf)

    common = {
        "source": np.ascontiguousarray(src.astype(bf)),
        "sourceT": np.ascontiguousarray(src.T.astype(bf)),
        "w1qT": np.ascontiguousarray(w1[:, :D].T.astype(bf)),
        "w1kT": np.ascontiguousarray(w1[:, D:].T.astype(bf)),
        "constf32": cf32,
        "constbf16": cbf,
        "waT": np.ascontiguousarray(wa.T.astype(bf)),

        "wbT": np.ascontiguousarray(wb.T.astype(bf)),

    }
    in_maps = []
    for i in range(NCORES):
        m = dict(common)
        m["x_shard"] = np.ascontiguousarray(x[:, QSH * i : QSH * (i + 1)].astype(bf))
        in_maps.append(m)
    return in_maps


def _assemble(results):
    y = np.zeros((1, D, NQ), np.float32)
    scores = np.zeros((1, NQ, NK), np.float32)
    for i, res in enumerate(results):
        y[0][:, QSH * i : QSH * (i + 1)] = res["y_sh"]
        scores[0][QSH * i : QSH * (i + 1), :] = res["scores_sh"]
    return y, scores


def run(inputs, trace=False, **kw):
    nc = _get_program()
    in_maps = _prepare_in_maps(**inputs)
    br = run_bass_kernel_spmd(nc, in_maps, core_ids=list(range(NCORES)),
                              trace=trace, **kw)
    y, scores = _assemble(br.results)
    return y, scores, br


def kernel(**inputs):
    y, scores, _ = run(inputs, trace=False)
    return y, scores


# revision 19
# speedup vs baseline: 1.0486x; 1.0486x over previous
"""AttentionalPropagation (SuperGlue-style MLP attention) Trainium2 kernel.

Full inputs in, full outputs out. Internally shards the nq (query) dimension
across 8 NeuronCores; source/keys and all MLP weights are replicated.

Math (per core, 64-query shard):
  aq = w1q @ q + b1            [256, 64]
  ak = w1k @ source            [256, 512]
  scores[n,m] = w2 . relu(aq[:,n] + ak[:,m]) + b2     (pairwise MLP score)
  prob = softmax_m(scores); message = source @ prob^T
  y = wb @ relu(wa @ [x; message] + ba) + bb

Implementation notes:
  - The pairwise tensor H_q = relu(ak + aq[:,q]) ([128, 512] per query per
    d-half, bf16) is built with single fused ops split across VectorE
    (tensor_scalar add+max), ScalarE (activation Relu+bias) and GpSimd.
  - The d-reduction (w2 . H) runs on the TensorEngine in bf16. To get score
    rows DENSELY packed in PSUM (DMA can't read PSUM and DVE/ACT cost is
    free-dim based), query r of a 16-query group uses lhsT = [0,..,0,w2half]
    (r+1 cols) so its row lands at partition base+r; zero columns add 0 to
    already-accumulated rows. 4 column-groups (tile_position) overlap on PE.
  - Softmax skips max-subtraction (scores are O(1), exp can't overflow).
  - prob (bf16) is transposed on the PE and the dense query columns gathered
    with a strided free-dim DVE copy; message + outer MLP are bf16 matmuls.
"""

import numpy as np
import ml_dtypes

import concourse.bass as bass
import concourse.bacc as bacc
import concourse.tile as tile
from concourse import mybir
from concourse.bass_utils import run_bass_kernel_spmd

D = 256
NQ = 512
NK = 512
NCORES = 8
QSH = NQ // NCORES      # queries per core = 64
NGRP = 4                # PE column groups
GQ = QSH // NGRP        # queries per group = 16
F32 = mybir.dt.float32
BF16 = mybir.dt.bfloat16

# construction engine pattern (per op index): D=vector, A=scalar, G=gpsimd
_ENG_PATTERN = "DDADDA"


def _build_program():
    nc = bacc.Bacc(trn_type="TRN2")

    # ---- DRAM parameters (per-core views; same program on all cores) ----
    d_src = nc.declare_dram_parameter("source", [D, NK], BF16, isOutput=False)
    d_w1kT = nc.declare_dram_parameter("w1kT", [D, D], BF16, isOutput=False)
    d_x = nc.declare_dram_parameter("x_shard", [D, QSH], BF16, isOutput=False)
    d_w1qT = nc.declare_dram_parameter("w1qT", [D, D], BF16, isOutput=False)
    d_cf32 = nc.declare_dram_parameter("constf32", [128, 9], F32, isOutput=False)
    d_cbf = nc.declare_dram_parameter("constbf16", [128, 2 * GQ + 128], BF16, isOutput=False)
    d_srcT = nc.declare_dram_parameter("sourceT", [NK, D], BF16, isOutput=False)
    d_waT = nc.declare_dram_parameter("waT", [2 * D, 2 * D], BF16, isOutput=False)
    d_wbT = nc.declare_dram_parameter("wbT", [2 * D, D], BF16, isOutput=False)
    d_scores = nc.declare_dram_parameter("scores_sh", [QSH, NK], F32, isOutput=True)
    d_y = nc.declare_dram_parameter("y_sh", [D, QSH], F32, isOutput=True)

    with tile.TileContext(nc) as tc:
        with (
            tc.tile_pool(name="consts", bufs=1) as consts,
            tc.tile_pool(name="acts", bufs=1) as acts,
            tc.tile_pool(name="hpool", bufs=24) as hpool,
            tc.tile_pool(name="psc", bufs=4, space="PSUM") as psc,
            tc.tile_pool(name="pmisc", bufs=4, space="PSUM") as pmisc,
        ):
            # ---------------- input loads (dependency order) ----------------
            src_sb = [consts.tile([128, NK], BF16, tag=f"src{c}", name=f"src{c}")
                      for c in range(2)]
            srcr = d_src[:].rearrange("(c p) m -> c p m", p=128)
            w1kT_sb = [consts.tile([128, D], BF16, tag=f"w1k{c}", name=f"w1k{c}")
                       for c in range(2)]
            w1kr = d_w1kT[:].rearrange("(c p) o -> c p o", p=128)
            for c in range(2):
                nc.sync.dma_start(out=w1kT_sb[c], in_=w1kr[c])
            for mh in range(2):
                for c in range(2):
                    nc.sync.dma_start(
                        out=src_sb[c][:, 256 * mh : 256 * (mh + 1)],
                        in_=srcr[c][:, 256 * mh : 256 * (mh + 1)],
                    )

            # cat = [x (2 chunks); message (2 chunks)]  as [128, 4, 64] bf16
            cat_sb = acts.tile([128, 4, QSH], BF16)
            xr = d_x[:].rearrange("(c p) n -> c p n", p=128)
            for c in range(2):
                nc.gpsimd.dma_start(out=cat_sb[:, c, :], in_=xr[c])

            w1qT_sb = [consts.tile([128, D], BF16, tag=f"w1q{c}", name=f"w1q{c}")
                       for c in range(2)]
            w1qr = d_w1qT[:].rearrange("(c p) o -> c p o", p=128)
            for c in range(2):
                nc.gpsimd.dma_start(out=w1qT_sb[c], in_=w1qr[c])

            # merged per-partition constants: cols 0-1 b1, 2 b2, 3-6 ba, 7-8 bb
            cf32_sb = consts.tile([128, 9], F32)
            nc.gpsimd.dma_start(out=cf32_sb, in_=d_cf32[:])
            b1_sb = cf32_sb[:, 0:2]
            b2_sb = cf32_sb[:, 2:3]
            ba_sb = cf32_sb[:, 3:7]
            bb_sb = cf32_sb[:, 7:9]
            # merged bf16 block: [w2pad (2*GQ) | eye128 (128)]
            cbf_sb = consts.tile([128, 2 * GQ + 128], BF16)
            nc.gpsimd.dma_start(out=cbf_sb, in_=d_cbf[:])
            w2p_sb = cbf_sb[:, 0 : 2 * GQ].rearrange("p (h g) -> p h g", h=2)
            eye_sb = cbf_sb[:, 2 * GQ : 2 * GQ + 128]

            srcT_sb = [consts.tile([128, D], BF16, tag=f"srcT{t}", name=f"srcT{t}")
                       for t in range(4)]
            srcTr = d_srcT[:].rearrange("(t p) d -> t p d", p=128)
            for t in range(4):
                nc.gpsimd.dma_start(out=srcT_sb[t], in_=srcTr[t])

            waT_sb = [consts.tile([128, 2 * D], BF16, tag=f"waT{c}", name=f"waT{c}")
                      for c in range(4)]
            waTr = d_waT[:].rearrange("(c p) o -> c p o", p=128)
            for c in range(4):
                nc.gpsimd.dma_start(out=waT_sb[c], in_=waTr[c])

            wbT_sb = [consts.tile([128, D], BF16, tag=f"wbT{c}", name=f"wbT{c}")
                      for c in range(4)]
            wbTr = d_wbT[:].rearrange("(c p) o -> c p o", p=128)
            for c in range(4):
                nc.gpsimd.dma_start(out=wbT_sb[c], in_=wbTr[c])

            # ---------------- ak / aq ----------------
            ak_sb = [acts.tile([128, NK], BF16, tag=f"ak{h}", name=f"ak{h}")
                     for h in range(2)]
            for oc in range(2):
                p_ak = pmisc.tile([128, NK], F32, tag="pmisc")
                for mh in range(2):
                    for kc in range(2):
                        nc.tensor.matmul(
                            out=p_ak[:, 256 * mh : 256 * (mh + 1)],
                            lhsT=w1kT_sb[kc][:, 128 * oc : 128 * (oc + 1)],
                            rhs=src_sb[kc][:, 256 * mh : 256 * (mh + 1)],
                            start=(mh == 0 and kc == 0),
                            stop=(mh == 1 and kc == 1),
                            skip_group_check=True,
                        )
                nc.scalar.copy(out=ak_sb[oc][:], in_=p_ak[:])

            aq_sb = [acts.tile([128, QSH], F32, tag=f"aq{h}", name=f"aq{h}")
                     for h in range(2)]
            for oc in range(2):
                p_aq = pmisc.tile([128, QSH], F32, tag="pmisc")
                for kc in range(2):
                    nc.tensor.matmul(
                        out=p_aq[:],
                        lhsT=w1qT_sb[kc][:, 128 * oc : 128 * (oc + 1)],
                        rhs=cat_sb[:, kc, :],
                        start=(kc == 0),
                        stop=(kc == 1),
                    )
                nc.vector.tensor_scalar(
                    out=aq_sb[oc][:],
                    in0=p_aq[:],
                    scalar1=b1_sb[:, oc : oc + 1],
                    scalar2=None,
                    op0=mybir.AluOpType.add,
                )

            # ---------------- pairwise scores ----------------
            # sc_sb row 32*g + r holds scores for query q = 16*g + r
            sc_sb = acts.tile([128, NK], F32)

            p_sc = [psc.tile([128, NK], F32, tag="psc", name=f"psc{g}")
                    for g in range(NGRP)]

            opi = 0
            for r in range(GQ - 1, -1, -1):
                for h in range(2):
                    hts = []
                    for g in range(NGRP):
                        q = GQ * g + r
                        ht = hpool.tile([128, NK], BF16, tag="H", name=f"H{opi}")
                        eng = _ENG_PATTERN[opi % len(_ENG_PATTERN)]
                        if eng == "A":
                            nc.scalar.activation(
                                out=ht[:],
                                in_=ak_sb[h][:],
                                func=mybir.ActivationFunctionType.Relu,
                                bias=aq_sb[h][:, q : q + 1],
                            )
                        elif eng == "G":
                            nc.gpsimd.tensor_scalar(
                                out=ht[:],
                                in0=ak_sb[h][:],
                                scalar1=aq_sb[h][:, q : q + 1],
                                scalar2=0.0,
                                op0=mybir.AluOpType.add,
                                op1=mybir.AluOpType.max,
                            )
                        else:
                            nc.vector.tensor_scalar(
                                out=ht[:],
                                in0=ak_sb[h][:],
                                scalar1=aq_sb[h][:, q : q + 1],
                                scalar2=0.0,
                                op0=mybir.AluOpType.add,
                                op1=mybir.AluOpType.max,
                            )
                        opi += 1
                        hts.append(ht)
                    for g in range(NGRP):
                        nc.tensor.matmul(
                            out=p_sc[g][32 * g : 32 * g + r + 1, :],
                            lhsT=w2p_sb[:, h, GQ - 1 - r : GQ],
                            rhs=hts[g][:],
                            start=(r == GQ - 1 and h == 0),
                            stop=(r == 0 and h == 1),
                            tile_position=(0, 32 * g),
                            skip_group_check=True,
                        )

            # fused per-group extraction: e = exp(scores + b2), accum -> sums
            e_sb = acts.tile([128, NK], F32)
            sums = acts.tile([128, 1], F32)
            nc.gpsimd.memset(e_sb, 0.0)
            nc.gpsimd.memset(sums, 1.0)
            for g in range(NGRP):
                lo = 32 * g
                nc.scalar.activation(
                    out=e_sb[lo : lo + GQ, :],
                    in_=p_sc[g][lo : lo + GQ, :],
                    func=mybir.ActivationFunctionType.Exp,
                    bias=b2_sb[lo : lo + GQ, :],
                    accum_out=sums[lo : lo + GQ, :],
                )

            # MLP1 x-part: h2 += waT[0:2].T @ x while softmax runs
            p_h2 = [psc.tile([128, QSH], F32, tag="psc", name=f"ph2{oc}")
                    for oc in range(4)]
            for oc in range(4):
                for kc in range(2):
                    nc.tensor.matmul(
                        out=p_h2[oc][:],
                        lhsT=waT_sb[kc][:, 128 * oc : 128 * (oc + 1)],
                        rhs=cat_sb[:, kc, :],
                        start=(kc == 0),
                        stop=False,
                        skip_group_check=True,
                    )

            # ---------------- softmax (no max-subtraction) ----------------
            rec = acts.tile([128, 1], F32)
            nc.vector.reciprocal(out=rec[:], in_=sums[:])
            prob = acts.tile([128, NK], BF16)
            nc.vector.tensor_scalar(
                out=prob[:], in0=e_sb[:], scalar1=rec[:, 0:1], scalar2=None,
                op0=mybir.AluOpType.mult,
            )

            # raw scores to DRAM (off the critical path)
            for g in range(NGRP):
                lo = 32 * g
                nc.scalar.activation(
                    out=sc_sb[lo : lo + GQ, :],
                    in_=p_sc[g][lo : lo + GQ, :],
                    func=mybir.ActivationFunctionType.Identity,
                    bias=b2_sb[lo : lo + GQ, :],
                )
                nc.sync.dma_start(
                    out=d_scores[:][GQ * g : GQ * (g + 1), :],
                    in_=sc_sb[lo : lo + GQ, :],
                )

            # ---------------- transpose prob, gather dense columns ----------
            probT = [acts.tile([128, NGRP, GQ], BF16, tag=f"pT{t}", name=f"pT{t}")
                     for t in range(4)]
            for mt in range(4):
                p_t = pmisc.tile([128, 128], BF16, tag="pmisc")
                nc.tensor.transpose(
                    out=p_t[:], in_=prob[:, 128 * mt : 128 * (mt + 1)],
                    identity=eye_sb[:],
                )
                nc.vector.tensor_copy(
                    out=probT[mt][:],
                    in_=p_t[:].rearrange("p (g x) -> p g x", g=4)[:, :, 0:GQ],
                )

            # ---------------- message = source @ prob^T ----------------
            for oc in range(2):
                p_msg = pmisc.tile([128, QSH], F32, tag="pmisc")
                for mtile in range(4):
                    nc.tensor.matmul(
                        out=p_msg[:],
                        lhsT=srcT_sb[mtile][:, 128 * oc : 128 * (oc + 1)],
                        rhs=probT[mtile][:],
                        start=(mtile == 0),
                        stop=(mtile == 3),
                    )
                nc.vector.tensor_copy(out=cat_sb[:, 2 + oc, :], in_=p_msg[:])

            # ---------------- outer MLP (message part accumulates) ----------
            h2_sb = acts.tile([128, 4, QSH], BF16)
            for oc in range(4):
                for kc in range(2, 4):
                    nc.tensor.matmul(
                        out=p_h2[oc][:],
                        lhsT=waT_sb[kc][:, 128 * oc : 128 * (oc + 1)],
                        rhs=cat_sb[:, kc, :],
                        start=False,
                        stop=(kc == 3),
                        skip_group_check=True,
                    )
                nc.vector.tensor_scalar(
                    out=h2_sb[:, oc, :], in0=p_h2[oc][:],
                    scalar1=ba_sb[:, oc : oc + 1], scalar2=0.0,
                    op0=mybir.AluOpType.add, op1=mybir.AluOpType.max,
                )

            yr = d_y[:].rearrange("(c p) n -> c p n", p=128)
            for oc in range(2):
                p_y = pmisc.tile([128, QSH], F32, tag="pmisc")
                for kc in range(4):
                    nc.tensor.matmul(
                        out=p_y[:],
                        lhsT=wbT_sb[kc][:, 128 * oc : 128 * (oc + 1)],
                        rhs=h2_sb[:, kc, :],
                        start=(kc == 0),
                        stop=(kc == 3),
                    )
                y_sb = acts.tile([128, QSH], F32, tag=f"y{oc}", name=f"ysb{oc}")
                nc.vector.tensor_scalar(
                    out=y_sb[:], in0=p_y[:], scalar1=bb_sb[:, oc : oc + 1],
                    scalar2=None, op0=mybir.AluOpType.add,
                )
                nc.sync.dma_start(out=yr[oc], in_=y_sb[:])

    nc.finalize()
    return nc


_NC_CACHE = None


def _get_program():
    global _NC_CACHE
    if _NC_CACHE is None:
        _NC_CACHE = _build_program()
    return _NC_CACHE


def _prepare_in_maps(x, source, w1, b1, w2, b2, wa, ba, wb, bb):
    f = np.float32
    bf = ml_dtypes.bfloat16
    x = np.asarray(x, f)[0]          # [D, NQ]
    src = np.asarray(source, f)[0]   # [D, NK]
    w1 = np.asarray(w1, f)
    b1 = np.asarray(b1, f)
    w2 = np.asarray(w2, f)
    b2 = np.asarray(b2, f)
    wa = np.asarray(wa, f)
    ba = np.asarray(ba, f)
    wb = np.asarray(wb, f)
    bb = np.asarray(bb, f)

    w2pad = np.zeros((128, 2, GQ), bf)
    for h in range(2):
        w2pad[:, h, GQ - 1] = w2[0, 128 * h : 128 * (h + 1)].astype(bf)
    # merged consts: f32 [128, 9] = [b1(2) | b2 | ba(4) | bb(2)]
    cf32 = np.zeros((128, 9), f)
    cf32[:, 0] = b1[:128]
    cf32[:, 1] = b1[128:]
    cf32[:, 2] = b2[0]
    for c in range(4):
        cf32[:, 3 + c] = ba[128 * c : 128 * (c + 1)]
    cf32[:, 7] = bb[:128]
    cf32[:, 8] = bb[128:]
    # merged bf16 [128, 2*GQ + 128] = [w2pad flat | eye128]
    cbf = np.zeros((128, 2 * GQ + 128), bf)
    cbf[:, : 2 * GQ] = w2pad.reshape(128, 2 * GQ)
    cbf[:, 2 * GQ :] = np.eye(128, dtype=bf)

    common = {
        "source": np.ascontiguousarray(src.astype(bf)),
        "sourceT": np.ascontiguousarray(src.T.astype(bf)),
        "w1qT": np.ascontiguousarray(w1[:, :D].T.astype(bf)),
        "w1kT": np.ascontiguousarray(w1[:, D:].T.astype(bf)),
        "constf32": cf32,
        "constbf16": cbf,
        "waT": np.ascontiguousarray(wa.T.astype(bf)),

        "wbT": np.ascontiguousarray(wb.T.astype(bf)),

    }
    in_maps = []
    for i in range(NCORES):
        m = dict(common)
        m["x_shard"] = np.ascontiguousarray(x[:, QSH * i : QSH * (i + 1)].astype(bf))
        in_maps.append(m)
    return in_maps


def _assemble(results):
    y = np.zeros((1, D, NQ), np.float32)
    scores = np.zeros((1, NQ, NK), np.float32)
    for i, res in enumerate(results):
        y[0][:, QSH * i : QSH * (i + 1)] = res["y_sh"]
        scores[0][QSH * i : QSH * (i + 1), :] = res["scores_sh"]
    return y, scores


def run(inputs, trace=False, **kw):
    nc = _get_program()
    in_maps = _prepare_in_maps(**inputs)
    br = run_bass_kernel_spmd(nc, in_maps, core_ids=list(range(NCORES)),
                              trace=trace, **kw)
    y, scores = _assemble(br.results)
    return y, scores, br


def kernel(**inputs):
    y, scores, _ = run(inputs, trace=False)
    return y, scores


# revision 20
# speedup vs baseline: 1.0701x; 1.0205x over previous
"""AttentionalPropagation (SuperGlue-style MLP attention) Trainium2 kernel.

Full inputs in, full outputs out. Internally shards the nq (query) dimension
across 8 NeuronCores; source/keys and all MLP weights are replicated.

Math (per core, 64-query shard):
  aq = w1q @ q + b1            [256, 64]
  ak = w1k @ source            [256, 512]
  scores[n,m] = w2 . relu(aq[:,n] + ak[:,m]) + b2     (pairwise MLP score)
  prob = softmax_m(scores); message = source @ prob^T
  y = wb @ relu(wa @ [x; message] + ba) + bb

Implementation notes:
  - The pairwise tensor H_q = relu(ak + aq[:,q]) ([128, 512] per query per
    d-half, bf16) is built with single fused ops split across VectorE
    (tensor_scalar add+max), ScalarE (activation Relu+bias) and GpSimd.
  - The d-reduction (w2 . H) runs on the TensorEngine in bf16. To get score
    rows DENSELY packed in PSUM (DMA can't read PSUM and DVE/ACT cost is
    free-dim based), query r of a 16-query group uses lhsT = [0,..,0,w2half]
    (r+1 cols) so its row lands at partition base+r; zero columns add 0 to
    already-accumulated rows. 4 column-groups (tile_position) overlap on PE.
  - Softmax skips max-subtraction (scores are O(1), exp can't overflow).
  - prob (bf16) is transposed on the PE and the dense query columns gathered
    with a strided free-dim DVE copy; message + outer MLP are bf16 matmuls.
"""

import numpy as np
import ml_dtypes

import concourse.bass as bass
import concourse.bacc as bacc
import concourse.tile as tile
from concourse import mybir
from concourse.bass_utils import run_bass_kernel_spmd

D = 256
NQ = 512
NK = 512
NCORES = 8
QSH = NQ // NCORES      # queries per core = 64
NGRP = 4                # PE column groups
GQ = QSH // NGRP        # queries per group = 16
F32 = mybir.dt.float32
BF16 = mybir.dt.bfloat16

# construction engine pattern (per op index): D=vector, A=scalar, G=gpsimd
_ENG_PATTERN = "DDADDA"


def _build_program():
    nc = bacc.Bacc(trn_type="TRN2", num_swdge_queues=4)

    # ---- DRAM parameters (per-core views; same program on all cores) ----
    d_src = nc.declare_dram_parameter("source", [D, NK], BF16, isOutput=False)
    d_w1kT = nc.declare_dram_parameter("w1kT", [D, D], BF16, isOutput=False)
    d_x = nc.declare_dram_parameter("x_shard", [D, QSH], BF16, isOutput=False)
    d_w1qT = nc.declare_dram_parameter("w1qT", [D, D], BF16, isOutput=False)
    d_cf32 = nc.declare_dram_parameter("constf32", [128, 9], F32, isOutput=False)
    d_cbf = nc.declare_dram_parameter("constbf16", [128, 2 * GQ + 128], BF16, isOutput=False)
    d_srcT = nc.declare_dram_parameter("sourceT", [NK, D], BF16, isOutput=False)
    d_waT = nc.declare_dram_parameter("waT", [2 * D, 2 * D], BF16, isOutput=False)
    d_wbT = nc.declare_dram_parameter("wbT", [2 * D, D], BF16, isOutput=False)
    d_scores = nc.declare_dram_parameter("scores_sh", [QSH, NK], F32, isOutput=True)
    d_y = nc.declare_dram_parameter("y_sh", [D, QSH], F32, isOutput=True)

    with tile.TileContext(nc) as tc:
        with (
            tc.tile_pool(name="consts", bufs=1) as consts,
            tc.tile_pool(name="acts", bufs=1) as acts,
            tc.tile_pool(name="hpool", bufs=24) as hpool,
            tc.tile_pool(name="psc", bufs=4, space="PSUM") as psc,
            tc.tile_pool(name="pmisc", bufs=4, space="PSUM") as pmisc,
        ):
            # ---------------- input loads (dependency order) ----------------
            src_sb = [consts.tile([128, NK], BF16, tag=f"src{c}", name=f"src{c}")
                      for c in range(2)]
            srcr = d_src[:].rearrange("(c p) m -> c p m", p=128)
            w1kT_sb = [consts.tile([128, D], BF16, tag=f"w1k{c}", name=f"w1k{c}")
                       for c in range(2)]
            w1kr = d_w1kT[:].rearrange("(c p) o -> c p o", p=128)
            for c in range(2):
                nc.sync.dma_start(out=w1kT_sb[c], in_=w1kr[c])
            for mh in range(2):
                for c in range(2):
                    nc.sync.dma_start(
                        out=src_sb[c][:, 256 * mh : 256 * (mh + 1)],
                        in_=srcr[c][:, 256 * mh : 256 * (mh + 1)],
                    )

            # cat = [x (2 chunks); message (2 chunks)]  as [128, 4, 64] bf16
            cat_sb = acts.tile([128, 4, QSH], BF16)
            xr = d_x[:].rearrange("(c p) n -> c p n", p=128)
            for c in range(2):
                nc.gpsimd.dma_start(out=cat_sb[:, c, :], in_=xr[c])

            w1qT_sb = [consts.tile([128, D], BF16, tag=f"w1q{c}", name=f"w1q{c}")
                       for c in range(2)]
            w1qr = d_w1qT[:].rearrange("(c p) o -> c p o", p=128)
            for c in range(2):
                nc.gpsimd.dma_start(out=w1qT_sb[c], in_=w1qr[c])

            # merged per-partition constants: cols 0-1 b1, 2 b2, 3-6 ba, 7-8 bb
            cf32_sb = consts.tile([128, 9], F32)
            nc.gpsimd.dma_start(out=cf32_sb, in_=d_cf32[:])
            b1_sb = cf32_sb[:, 0:2]
            b2_sb = cf32_sb[:, 2:3]
            ba_sb = cf32_sb[:, 3:7]
            bb_sb = cf32_sb[:, 7:9]
            # merged bf16 block: [w2pad (2*GQ) | eye128 (128)]
            cbf_sb = consts.tile([128, 2 * GQ + 128], BF16)
            nc.gpsimd.dma_start(out=cbf_sb, in_=d_cbf[:])
            w2p_sb = cbf_sb[:, 0 : 2 * GQ].rearrange("p (h g) -> p h g", h=2)
            eye_sb = cbf_sb[:, 2 * GQ : 2 * GQ + 128]

            srcT_sb = [consts.tile([128, D], BF16, tag=f"srcT{t}", name=f"srcT{t}")
                       for t in range(4)]
            srcTr = d_srcT[:].rearrange("(t p) d -> t p d", p=128)
            for t in range(4):
                nc.gpsimd.dma_start(out=srcT_sb[t], in_=srcTr[t])

            waT_sb = [consts.tile([128, 2 * D], BF16, tag=f"waT{c}", name=f"waT{c}")
                      for c in range(4)]
            waTr = d_waT[:].rearrange("(c p) o -> c p o", p=128)
            for c in range(4):
                nc.gpsimd.dma_start(out=waT_sb[c], in_=waTr[c])

            wbT_sb = [consts.tile([128, D], BF16, tag=f"wbT{c}", name=f"wbT{c}")
                      for c in range(4)]
            wbTr = d_wbT[:].rearrange("(c p) o -> c p o", p=128)
            for c in range(4):
                nc.gpsimd.dma_start(out=wbT_sb[c], in_=wbTr[c])

            # ---------------- ak / aq ----------------
            ak_sb = [acts.tile([128, NK], BF16, tag=f"ak{h}", name=f"ak{h}")
                     for h in range(2)]
            for oc in range(2):
                p_ak = pmisc.tile([128, NK], F32, tag="pmisc")
                for mh in range(2):
                    for kc in range(2):
                        nc.tensor.matmul(
                            out=p_ak[:, 256 * mh : 256 * (mh + 1)],
                            lhsT=w1kT_sb[kc][:, 128 * oc : 128 * (oc + 1)],
                            rhs=src_sb[kc][:, 256 * mh : 256 * (mh + 1)],
                            start=(mh == 0 and kc == 0),
                            stop=(mh == 1 and kc == 1),
                            skip_group_check=True,
                        )
                nc.scalar.copy(out=ak_sb[oc][:], in_=p_ak[:])

            aq_sb = [acts.tile([128, QSH], F32, tag=f"aq{h}", name=f"aq{h}")
                     for h in range(2)]
            for oc in range(2):
                p_aq = pmisc.tile([128, QSH], F32, tag="pmisc")
                for kc in range(2):
                    nc.tensor.matmul(
                        out=p_aq[:],
                        lhsT=w1qT_sb[kc][:, 128 * oc : 128 * (oc + 1)],
                        rhs=cat_sb[:, kc, :],
                        start=(kc == 0),
                        stop=(kc == 1),
                    )
                nc.vector.tensor_scalar(
                    out=aq_sb[oc][:],
                    in0=p_aq[:],
                    scalar1=b1_sb[:, oc : oc + 1],
                    scalar2=None,
                    op0=mybir.AluOpType.add,
                )

            # ---------------- pairwise scores ----------------
            # sc_sb row 32*g + r holds scores for query q = 16*g + r
            sc_sb = acts.tile([128, NK], F32)

            p_sc = [psc.tile([128, NK], F32, tag="psc", name=f"psc{g}")
                    for g in range(NGRP)]

            opi = 0
            for r in range(GQ - 1, -1, -1):
                for h in range(2):
                    hts = []
                    for g in range(NGRP):
                        q = GQ * g + r
                        ht = hpool.tile([128, NK], BF16, tag="H", name=f"H{opi}")
                        eng = _ENG_PATTERN[opi % len(_ENG_PATTERN)]
                        if eng == "A":
                            nc.scalar.activation(
                                out=ht[:],
                                in_=ak_sb[h][:],
                                func=mybir.ActivationFunctionType.Relu,
                                bias=aq_sb[h][:, q : q + 1],
                            )
                        elif eng == "G":
                            nc.gpsimd.tensor_scalar(
                                out=ht[:],
                                in0=ak_sb[h][:],
                                scalar1=aq_sb[h][:, q : q + 1],
                                scalar2=0.0,
                                op0=mybir.AluOpType.add,
                                op1=mybir.AluOpType.max,
                            )
                        else:
                            nc.vector.tensor_scalar(
                                out=ht[:],
                                in0=ak_sb[h][:],
                                scalar1=aq_sb[h][:, q : q + 1],
                                scalar2=0.0,
                                op0=mybir.AluOpType.add,
                                op1=mybir.AluOpType.max,
                            )
                        opi += 1
                        hts.append(ht)
                    for g in range(NGRP):
                        nc.tensor.matmul(
                            out=p_sc[g][32 * g : 32 * g + r + 1, :],
                            lhsT=w2p_sb[:, h, GQ - 1 - r : GQ],
                            rhs=hts[g][:],
                            start=(r == GQ - 1 and h == 0),
                            stop=(r == 0 and h == 1),
                            tile_position=(0, 32 * g),
                            skip_group_check=True,
                        )

            # fused per-group extraction: e = exp(scores + b2), accum -> sums
            e_sb = acts.tile([128, NK], F32)
            sums = acts.tile([128, 1], F32)
            nc.gpsimd.memset(e_sb, 0.0)
            nc.gpsimd.memset(sums, 1.0)
            for g in range(NGRP):
                lo = 32 * g
                nc.scalar.activation(
                    out=e_sb[lo : lo + GQ, :],
                    in_=p_sc[g][lo : lo + GQ, :],
                    func=mybir.ActivationFunctionType.Exp,
                    bias=b2_sb[lo : lo + GQ, :],
                    accum_out=sums[lo : lo + GQ, :],
                )

            # MLP1 x-part: h2 += waT[0:2].T @ x while softmax runs
            p_h2 = [psc.tile([128, QSH], F32, tag="psc", name=f"ph2{oc}")
                    for oc in range(4)]
            for oc in range(4):
                for kc in range(2):
                    nc.tensor.matmul(
                        out=p_h2[oc][:],
                        lhsT=waT_sb[kc][:, 128 * oc : 128 * (oc + 1)],
                        rhs=cat_sb[:, kc, :],
                        start=(kc == 0),
                        stop=False,
                        skip_group_check=True,
                    )

            # ---------------- softmax (no max-subtraction) ----------------
            rec = acts.tile([128, 1], F32)
            nc.vector.reciprocal(out=rec[:], in_=sums[:])
            prob = acts.tile([128, NK], BF16)
            nc.vector.tensor_scalar(
                out=prob[:], in0=e_sb[:], scalar1=rec[:, 0:1], scalar2=None,
                op0=mybir.AluOpType.mult,
            )

            # raw scores to DRAM (off the critical path)
            for g in range(NGRP):
                lo = 32 * g
                nc.scalar.activation(
                    out=sc_sb[lo : lo + GQ, :],
                    in_=p_sc[g][lo : lo + GQ, :],
                    func=mybir.ActivationFunctionType.Identity,
                    bias=b2_sb[lo : lo + GQ, :],
                )
                nc.sync.dma_start(
                    out=d_scores[:][GQ * g : GQ * (g + 1), :],
                    in_=sc_sb[lo : lo + GQ, :],
                )

            # ---------------- transpose prob, gather dense columns ----------
            probT = [acts.tile([128, NGRP, GQ], BF16, tag=f"pT{t}", name=f"pT{t}")
                     for t in range(4)]
            for mt in range(4):
                p_t = pmisc.tile([128, 128], BF16, tag="pmisc")
                nc.tensor.transpose(
                    out=p_t[:], in_=prob[:, 128 * mt : 128 * (mt + 1)],
                    identity=eye_sb[:],
                )
                nc.vector.tensor_copy(
                    out=probT[mt][:],
                    in_=p_t[:].rearrange("p (g x) -> p g x", g=4)[:, :, 0:GQ],
                )

            # ---------------- message = source @ prob^T ----------------
            for oc in range(2):
                p_msg = pmisc.tile([128, QSH], F32, tag="pmisc")
                for mtile in range(4):
                    nc.tensor.matmul(
                        out=p_msg[:],
                        lhsT=srcT_sb[mtile][:, 128 * oc : 128 * (oc + 1)],
                        rhs=probT[mtile][:],
                        start=(mtile == 0),
                        stop=(mtile == 3),
                    )
                nc.vector.tensor_copy(out=cat_sb[:, 2 + oc, :], in_=p_msg[:])

            # ---------------- outer MLP (message part accumulates) ----------
            h2_sb = acts.tile([128, 4, QSH], BF16)
            for oc in range(4):
                for kc in range(2, 4):
                    nc.tensor.matmul(
                        out=p_h2[oc][:],
                        lhsT=waT_sb[kc][:, 128 * oc : 128 * (oc + 1)],
                        rhs=cat_sb[:, kc, :],
                        start=False,
                        stop=(kc == 3),
                        skip_group_check=True,
                    )
                if oc % 2 == 0:
                    nc.scalar.activation(
                        out=h2_sb[:, oc, :], in_=p_h2[oc][:],
                        func=mybir.ActivationFunctionType.Relu,
                        bias=ba_sb[:, oc : oc + 1],
                    )
                else:
                    nc.vector.tensor_scalar(
                        out=h2_sb[:, oc, :], in0=p_h2[oc][:],
                        scalar1=ba_sb[:, oc : oc + 1], scalar2=0.0,
                        op0=mybir.AluOpType.add, op1=mybir.AluOpType.max,
                    )

            yr = d_y[:].rearrange("(c p) n -> c p n", p=128)
            for oc in range(2):
                p_y = pmisc.tile([128, QSH], F32, tag="pmisc")
                for kc in range(4):
                    nc.tensor.matmul(
                        out=p_y[:],
                        lhsT=wbT_sb[kc][:, 128 * oc : 128 * (oc + 1)],
                        rhs=h2_sb[:, kc, :],
                        start=(kc == 0),
                        stop=(kc == 3),
                    )
                y_sb = acts.tile([128, QSH], F32, tag=f"y{oc}", name=f"ysb{oc}")
                if oc == 0:
                    nc.scalar.activation(
                        out=y_sb[:], in_=p_y[:],
                        func=mybir.ActivationFunctionType.Identity,
                        bias=bb_sb[:, oc : oc + 1],
                    )
                else:
                    nc.vector.tensor_scalar(
                        out=y_sb[:], in0=p_y[:], scalar1=bb_sb[:, oc : oc + 1],
                        scalar2=None, op0=mybir.AluOpType.add,
                    )
                nc.sync.dma_start(out=yr[oc], in_=y_sb[:])

    nc.finalize()
    return nc


_NC_CACHE = None


def _get_program():
    global _NC_CACHE
    if _NC_CACHE is None:
        _NC_CACHE = _build_program()
    return _NC_CACHE


def _prepare_in_maps(x, source, w1, b1, w2, b2, wa, ba, wb, bb):
    f = np.float32
    bf = ml_dtypes.bfloat16
    x = np.asarray(x, f)[0]          # [D, NQ]
    src = np.asarray(source, f)[0]   # [D, NK]
    w1 = np.asarray(w1, f)
    b1 = np.asarray(b1, f)
    w2 = np.asarray(w2, f)
    b2 = np.asarray(b2, f)
    wa = np.asarray(wa, f)
    ba = np.asarray(ba, f)
    wb = np.asarray(wb, f)
    bb = np.asarray(bb, f)

    w2pad = np.zeros((128, 2, GQ), bf)
    for h in range(2):
        w2pad[:, h, GQ - 1] = w2[0, 128 * h : 128 * (h + 1)].astype(bf)
    # merged consts: f32 [128, 9] = [b1(2) | b2 | ba(4) | bb(2)]
    cf32 = np.zeros((128, 9), f)
    cf32[:, 0] = b1[:128]
    cf32[:, 1] = b1[128:]
    cf32[:, 2] = b2[0]
    for c in range(4):
        cf32[:, 3 + c] = ba[128 * c : 128 * (c + 1)]
    cf32[:, 7] = bb[:128]
    cf32[:, 8] = bb[128:]
    # merged bf16 [128, 2*GQ + 128] = [w2pad flat | eye128]
    cbf = np.zeros((128, 2 * GQ + 128), bf)
    cbf[:, : 2 * GQ] = w2pad.reshape(128, 2 * GQ)
    cbf[:, 2 * GQ :] = np.eye(128, dtype=bf)

    common = {
        "source": np.ascontiguousarray(src.astype(bf)),
        "sourceT": np.ascontiguousarray(src.T.astype(bf)),
        "w1qT": np.ascontiguousarray(w1[:, :D].T.astype(bf)),
        "w1kT": np.ascontiguousarray(w1[:, D:].T.astype(bf)),
        "constf32": cf32,
        "constbf16": cbf,
        "waT": np.ascontiguousarray(wa.T.astype(bf)),

        "wbT": np.ascontiguousarray(wb.T.astype(bf)),

    }
    in_maps = []
    for i in range(NCORES):
        m = dict(common)
        m["x_shard"] = np.ascontiguousarray(x[:, QSH * i : QSH * (i + 1)].astype(bf))
        in_maps.append(m)
    return in_maps


def _assemble(results):
    y = np.zeros((1, D, NQ), np.float32)
    scores = np.zeros((1, NQ, NK), np.float32)
    for i, res in enumerate(results):
        y[0][:, QSH * i : QSH * (i + 1)] = res["y_sh"]
        scores[0][QSH * i : QSH * (i + 1), :] = res["scores_sh"]
    return y, scores


def run(inputs, trace=False, **kw):
    nc = _get_program()
    in_maps = _prepare_in_maps(**inputs)
    br = run_bass_kernel_spmd(nc, in_maps, core_ids=list(range(NCORES)),
                              trace=trace, **kw)
    y, scores = _assemble(br.results)
    return y, scores, br


def kernel(**inputs):
    y, scores, _ = run(inputs, trace=False)
    return y, scores
